# revision 1
# baseline (speedup 1.0000x reference)
"""Multi-head attention (B=2, S=2048, D=1024, H=16) on 8 NeuronCores.

Sharding: Megatron tensor parallelism. Core r owns heads 2r, 2r+1
(a 128-wide slice of D). Wq/Wk/Wv column-parallel, Wo row-parallel,
ReduceScatter(add) over tokens at the end; host concatenates the 8
token slices and adds bo.

Layouts on device (per core):
  xqT/xkT/xvT : [1024, 4096]  host-transposed activations (feature-major)
  qT/kT       : [128, 2048]   per batch, dk-major (rows = this core's 2 heads)
  v           : [128, 130]    16 token-tiles per batch; cols = [v_h0 | 1 | v_h1 | 1]
                              (ones column makes the PV matmul emit softmax sums)
  scores^T    : psum [128 sk, 512 sq] -> exp on ACT -> PT sbuf
  PV          : psum [65, 512] accumulated over 16 sk tiles; row 64 = sums
  attnT       : [128, 2048]   per batch, normalized, = lhsT for Wo matmul
"""

import sys

sys.path.insert(0, "/opt/trn_rl_repo")

import numpy as np

B, S, D, H, DK = 2, 2048, 1024, 16, 64
NCORES = 8
TOK = B * S            # 4096
DKC = D // NCORES      # 128 = 2 heads per core
TOKC = TOK // NCORES   # 512 output rows per core
KT = D // 128          # 8 contraction tiles
SKT = S // 128         # 16 key tiles per batch
SQB = S // 512         # 4 query blocks per batch

# matmul operand dtype: float32 (exact, 4 cyc/row) or float32r (1 cyc/row)
MM_DT_NAME = "float32r"

_cache = {}


def _build(collective=True):
    from contextlib import ExitStack

    from concourse import bacc
    import concourse.mybir as mybir
    import concourse.tile as tile

    f32 = mybir.dt.float32
    mm_dt = getattr(mybir.dt, MM_DT_NAME)
    Act = mybir.ActivationFunctionType

    def c(ap):
        # bitcast DRAM sources feeding matmul-operand tiles to the matmul dtype
        return ap.bitcast(mm_dt) if mm_dt != f32 else ap

    nc = bacc.Bacc(
        "TRN2", target_bir_lowering=False, debug=False,
        enable_asserts=False, num_devices=NCORES,
    )

    xqT = nc.dram_tensor("xqT", [D, TOK], f32, kind="ExternalInput").ap()
    xkT = nc.dram_tensor("xkT", [D, TOK], f32, kind="ExternalInput").ap()
    xvT = nc.dram_tensor("xvT", [D, TOK], f32, kind="ExternalInput").ap()
    wq = nc.dram_tensor("wq", [D, DKC], f32, kind="ExternalInput").ap()
    wk = nc.dram_tensor("wk", [D, DKC], f32, kind="ExternalInput").ap()
    wv = nc.dram_tensor("wv", [D, DKC], f32, kind="ExternalInput").ap()
    wo = nc.dram_tensor("wo", [DKC, D], f32, kind="ExternalInput").ap()
    bq = nc.dram_tensor("bq", [DKC, 1], f32, kind="ExternalInput").ap()
    bk = nc.dram_tensor("bk", [DKC, 1], f32, kind="ExternalInput").ap()
    bv = nc.dram_tensor("bv", [1, DKC], f32, kind="ExternalInput").ap()
    out_ext = nc.dram_tensor("out", [TOKC, D], f32, kind="ExternalOutput").ap()

    with tile.TileContext(nc) as tc, ExitStack() as ctx, \
            nc.allow_low_precision("float32r matmul operands, fp32 psum accumulate"):
        wpool = ctx.enter_context(tc.tile_pool(name="w", bufs=1))
        xpool = ctx.enter_context(tc.tile_pool(name="x", bufs=12))
        qkpool = ctx.enter_context(tc.tile_pool(name="qk", bufs=2))
        vpool = ctx.enter_context(tc.tile_pool(name="v", bufs=32))
        ptpool = ctx.enter_context(tc.tile_pool(name="pt", bufs=6))
        atpool = ctx.enter_context(tc.tile_pool(name="at", bufs=2))
        smpool = ctx.enter_context(tc.tile_pool(name="sm", bufs=4))
        opool = ctx.enter_context(tc.tile_pool(name="o", bufs=4))
        ps_mm = ctx.enter_context(tc.tile_pool(name="psmm", bufs=3, space="PSUM"))
        ps_acc = ctx.enter_context(tc.tile_pool(name="psacc", bufs=2, space="PSUM"))
        dram = ctx.enter_context(tc.tile_pool(name="dram", bufs=1, space="DRAM"))

        # ---- constants / weights into SBUF ----
        wq_t, wk_t, wv_t = [], [], []
        for name, src, lst in (("wq", wq, wq_t), ("wk", wk, wk_t), ("wv", wv, wv_t)):
            for k in range(KT):
                t = wpool.tile([128, DKC], mm_dt, tag=f"{name}{k}")
                nc.sync.dma_start(t[:], c(src[k * 128:(k + 1) * 128, :]))
                lst.append(t)
        wo_t = wpool.tile([DKC, D], mm_dt, tag="wo")
        nc.sync.dma_start(wo_t[:], c(wo[:]))
        bq_t = wpool.tile([DKC, 1], f32, tag="bq")
        nc.sync.dma_start(bq_t[:], bq[:])
        bk_t = wpool.tile([DKC, 1], f32, tag="bk")
        nc.sync.dma_start(bk_t[:], bk[:])
        bv_t = wpool.tile([1, DKC], mm_dt, tag="bv")
        nc.sync.dma_start(bv_t[:], c(bv[:]))
        ones_f = wpool.tile([1, 128], f32, tag="onesf")
        nc.gpsimd.memset(ones_f[:], 1.0)
        ones_t = wpool.tile([1, 128], mm_dt, tag="ones")
        nc.vector.tensor_copy(ones_t[:], ones_f[:])
        onescol_f = wpool.tile([128, 1], f32, tag="onescolf")
        nc.gpsimd.memset(onescol_f[:], 1.0)

        partial = dram.tile([TOK, D], f32, tag="partial")
        rs_out = dram.tile([TOKC, D], f32, tag="rsout")

        for b in range(B):
            t0 = b * S
            # ---- q/k projections -> qT_b, kT_b [128, S] (dk-major) ----
            qT_b = qkpool.tile([128, S], mm_dt, tag="qT")
            kT_b = qkpool.tile([128, S], mm_dt, tag="kT")
            for xT, w_list, bias_t, dst in (
                (xqT, wq_t, bq_t, qT_b), (xkT, wk_t, bk_t, kT_b),
            ):
                for blk in range(SQB):
                    ps = ps_mm.tile([128, 512], f32, tag="mm")
                    for k in range(KT):
                        xt = xpool.tile([128, 512], mm_dt, tag="xt")
                        nc.sync.dma_start(
                            xt[:],
                            c(xT[k * 128:(k + 1) * 128,
                                 t0 + blk * 512: t0 + (blk + 1) * 512]),
                        )
                        nc.tensor.matmul(
                            ps[:], lhsT=w_list[k][:], rhs=xt[:],
                            start=(k == 0), stop=(k == KT - 1),
                        )
                    nc.scalar.activation(
                        dst[:, blk * 512:(blk + 1) * 512], ps[:],
                        Act.Identity, bias=bias_t[:, 0:1],
                    )

            # ---- v projection -> 16 tiles [128 tok, 130] ----
            v_tiles = []
            for blk in range(SQB):
                xv_blk = []
                for k in range(KT):
                    xt = xpool.tile([128, 512], mm_dt, tag="xt")
                    nc.sync.dma_start(
                        xt[:],
                        c(xvT[k * 128:(k + 1) * 128,
                              t0 + blk * 512: t0 + (blk + 1) * 512]),
                    )
                    xv_blk.append(xt)
                for mi in range(4):
                    ps = ps_mm.tile([128, DKC], f32, tag="mm")
                    for k in range(KT):
                        nc.tensor.matmul(
                            ps[:], lhsT=xv_blk[k][:, mi * 128:(mi + 1) * 128],
                            rhs=wv_t[k][:], start=(k == 0), stop=False,
                        )
                    nc.tensor.matmul(
                        ps[:], lhsT=ones_t[0:1, :], rhs=bv_t[:],
                        start=False, stop=True,
                    )
                    vt = vpool.tile([128, 130], mm_dt, tag="v")
                    nc.vector.tensor_copy(vt[:, 0:64], ps[:, 0:64])
                    nc.vector.tensor_copy(vt[:, 65:129], ps[:, 64:128])
                    nc.vector.tensor_copy(vt[:, 64:65], onescol_f[:])
                    nc.vector.tensor_copy(vt[:, 129:130], onescol_f[:])
                    v_tiles.append(vt)

            # ---- attention (2 heads) -> attnT_b [128, S] ----
            attnT_b = atpool.tile([128, S], mm_dt, tag="attnT")
            for h in range(2):
                hp = h * 64
                for sq in range(SQB):
                    qs = slice(sq * 512, (sq + 1) * 512)
                    xps = ps_acc.tile([65, 512], f32, tag="acc")
                    for sk in range(SKT):
                        sps = ps_mm.tile([128, 512], f32, tag="mm")
                        nc.tensor.matmul(
                            sps[:],
                            lhsT=kT_b[hp:hp + 64, sk * 128:(sk + 1) * 128],
                            rhs=qT_b[hp:hp + 64, qs],
                            start=True, stop=True,
                        )
                        pt = ptpool.tile([128, 512], mm_dt, tag="pt")
                        nc.scalar.activation(pt[:], sps[:], Act.Exp, scale=0.125)
                        nc.tensor.matmul(
                            xps[:], lhsT=v_tiles[sk][:, h * 65:h * 65 + 65],
                            rhs=pt[:], start=(sk == 0), stop=(sk == SKT - 1),
                        )
                    rec = smpool.tile([1, 512], mm_dt, tag="rec")
                    nc.vector.reciprocal(rec[:], xps[64:65, :])
                    rbp = ps_mm.tile([64, 512], f32, tag="mm")
                    nc.tensor.matmul(
                        rbp[:], lhsT=ones_t[0:1, 0:64], rhs=rec[:],
                        start=True, stop=True,
                    )
                    rb = smpool.tile([64, 512], f32, tag="rb")
                    nc.scalar.copy(rb[:], rbp[:])
                    nc.vector.tensor_mul(
                        attnT_b[hp:hp + 64, qs], xps[0:64, :], rb[:],
                    )

            # ---- output projection partial [S, D] ----
            for m in range(S // 128):
                for n2 in range(2):
                    ops = ps_mm.tile([128, 512], f32, tag="mm")
                    nc.tensor.matmul(
                        ops[:], lhsT=attnT_b[:, m * 128:(m + 1) * 128],
                        rhs=wo_t[:, n2 * 512:(n2 + 1) * 512],
                        start=True, stop=True,
                    )
                    ot = opool.tile([128, 512], f32, tag="ot")
                    nc.vector.tensor_copy(ot[:], ops[:])
                    nc.sync.dma_start(
                        partial[t0 + m * 128: t0 + (m + 1) * 128,
                                n2 * 512:(n2 + 1) * 512],
                        ot[:],
                    )

        if collective:
            nc.gpsimd.collective_compute(
                "ReduceScatter",
                mybir.AluOpType.add,
                replica_groups=[list(range(NCORES))],
                ins=[partial.opt()],
                outs=[rs_out.opt()],
            )
            nc.sync.dma_start(out_ext[:], rs_out[:])
        else:
            nc.sync.dma_start(out_ext[:], partial[0:TOKC, :])

    nc.compile()
    return nc


def _get_nc():
    if "nc" not in _cache:
        _cache["nc"] = _build()
    return _cache["nc"]


def kernel(query, key, value, Wq, bq, Wk, bk, Wv, bv, Wo, bo, trace=False):
    from concourse.bass_utils import run_bass_kernel_spmd

    nc = _get_nc()

    q = np.ascontiguousarray(np.asarray(query, np.float32).reshape(TOK, D).T)
    k = np.ascontiguousarray(np.asarray(key, np.float32).reshape(TOK, D).T)
    v = np.ascontiguousarray(np.asarray(value, np.float32).reshape(TOK, D).T)
    Wq = np.asarray(Wq, np.float32)
    Wk = np.asarray(Wk, np.float32)
    Wv = np.asarray(Wv, np.float32)
    Wo = np.asarray(Wo, np.float32)

    in_maps = []
    for r in range(NCORES):
        sl = slice(r * DKC, (r + 1) * DKC)
        in_maps.append({
            "xqT": q, "xkT": k, "xvT": v,
            "wq": np.ascontiguousarray(Wq[:, sl]),
            "wk": np.ascontiguousarray(Wk[:, sl]),
            "wv": np.ascontiguousarray(Wv[:, sl]),
            "wo": np.ascontiguousarray(Wo[sl, :]),
            "bq": np.ascontiguousarray(np.asarray(bq, np.float32)[sl, None]),
            "bk": np.ascontiguousarray(np.asarray(bk, np.float32)[sl, None]),
            "bv": np.ascontiguousarray(np.asarray(bv, np.float32)[None, sl]),
        })

    res = run_bass_kernel_spmd(nc, in_maps, list(range(NCORES)), trace=trace)
    _cache["last_results"] = res

    out = np.concatenate([res.results[r]["out"] for r in range(NCORES)], axis=0)
    out = out + np.asarray(bo, np.float32)[None, :]
    return out.reshape(B, S, D)



# revision 5
# speedup vs baseline: 1.5222x; 1.5222x over previous
"""Multi-head attention (B=2, S=2048, D=1024, H=16) on 8 NeuronCores.

Sharding: Megatron tensor parallelism. Core r owns heads 2r, 2r+1
(a 128-wide slice of D). Wq/Wk/Wv column-parallel. The output
projection is token-parallel: an AllToAll exchanges attnT feature
slices for token slices (1 MB fp16 per core, vs 16.8 MB for the
row-parallel ReduceScatter), then each core computes its 512-token
output rows with the full Wo. Host concatenates the 8 token slices.

All matmul operands are fp16 (1 cyc/row on the PE, no fp32r power
throttle, half the DMA); PSUM accumulation stays fp32.

Layouts on device (per core):
  xqT/xkT/xvT : [1024, 4096]  host-transposed fp16 activations
  qT/kT       : [128, 2048]   per batch, dk-major (rows = 2 heads)
  v           : [128, 130]    16 token-tiles per batch; cols = [v_h0|1|v_h1|1]
                              (ones column makes PV emit softmax sums)
  scores^T    : psum [128 sk, 512 sq] -> exp on ACT -> pt sbuf fp16
  PV          : psum [65, 512] accumulated over 16 sk tiles; row 64 = sums
  attnT       : [128, 2048]   per batch, normalized fp16
  a2a_src/dst : [1024, 512]   fp16; block c of src = my feats, c's tokens
  out         : [512, 1024]   fp32 rows for my token slice (bo added on dev)
"""

import sys

sys.path.insert(0, "/opt/trn_rl_repo")

import numpy as np

B, S, D, H, DK = 2, 2048, 1024, 16, 64
NCORES = 8
TOK = B * S            # 4096
DKC = D // NCORES      # 128 = 2 heads per core
TOKC = TOK // NCORES   # 512 output rows per core
KT = D // 128          # 8 contraction tiles
SKT = S // 128         # 16 key tiles per batch
SQB = S // 512         # 4 query blocks per batch

_cache = {}


def _build():
    from contextlib import ExitStack

    from concourse import bacc
    import concourse.mybir as mybir
    import concourse.tile as tile

    f32 = mybir.dt.float32
    f16 = mybir.dt.float16
    Act = mybir.ActivationFunctionType

    nc = bacc.Bacc(
        "TRN2", target_bir_lowering=False, debug=False,
        enable_asserts=False, num_devices=NCORES,
    )

    xqT = nc.dram_tensor("xqT", [D, TOK], f16, kind="ExternalInput").ap()
    xkT = nc.dram_tensor("xkT", [D, TOK], f16, kind="ExternalInput").ap()
    xvT = nc.dram_tensor("xvT", [D, TOK], f16, kind="ExternalInput").ap()
    wq = nc.dram_tensor("wq", [D, DKC], f16, kind="ExternalInput").ap()
    wk = nc.dram_tensor("wk", [D, DKC], f16, kind="ExternalInput").ap()
    wv = nc.dram_tensor("wv", [D, DKC], f16, kind="ExternalInput").ap()
    wo = nc.dram_tensor("wo", [D, D], f16, kind="ExternalInput").ap()
    bq = nc.dram_tensor("bq", [DKC, 1], f32, kind="ExternalInput").ap()
    bk = nc.dram_tensor("bk", [DKC, 1], f32, kind="ExternalInput").ap()
    bv = nc.dram_tensor("bv", [1, DKC], f16, kind="ExternalInput").ap()
    bo = nc.dram_tensor("bo", [1, D], f16, kind="ExternalInput").ap()
    out_ext = nc.dram_tensor("out", [TOKC, D], f32, kind="ExternalOutput").ap()

    with tile.TileContext(nc) as tc, ExitStack() as ctx, \
            nc.allow_low_precision("fp16 matmul operands, fp32 psum accumulate"):
        wpool = ctx.enter_context(tc.tile_pool(name="w", bufs=1))
        xpool = ctx.enter_context(tc.tile_pool(name="x", bufs=10))
        qkpool = ctx.enter_context(tc.tile_pool(name="qk", bufs=1))
        vpool = ctx.enter_context(tc.tile_pool(name="v", bufs=1))
        ptpool = ctx.enter_context(tc.tile_pool(name="pt", bufs=6))
        atpool = ctx.enter_context(tc.tile_pool(name="at", bufs=1))
        smpool = ctx.enter_context(tc.tile_pool(name="sm", bufs=4))
        opool = ctx.enter_context(tc.tile_pool(name="o", bufs=4))
        rpool = ctx.enter_context(tc.tile_pool(name="recv", bufs=1))
        ps_mm = ctx.enter_context(tc.tile_pool(name="psmm", bufs=3, space="PSUM"))
        ps_acc = ctx.enter_context(tc.tile_pool(name="psacc", bufs=2, space="PSUM"))
        dram = ctx.enter_context(tc.tile_pool(name="dram", bufs=1, space="DRAM"))

        # ---- constants / weights into SBUF ----
        wq_t, wk_t, wv_t = [], [], []
        for name, src, lst in (("wq", wq, wq_t), ("wk", wk, wk_t), ("wv", wv, wv_t)):
            for k in range(KT):
                t = wpool.tile([128, DKC], f16, tag=f"{name}{k}")
                nc.sync.dma_start(t[:], src[k * 128:(k + 1) * 128, :])
                lst.append(t)
        wo_t = []
        for k in range(KT):
            t = wpool.tile([128, D], f16, tag=f"wo{k}")
            nc.sync.dma_start(t[:], wo[k * 128:(k + 1) * 128, :])
            wo_t.append(t)
        bq_t = wpool.tile([DKC, 1], f32, tag="bq")
        nc.sync.dma_start(bq_t[:], bq[:])
        bk_t = wpool.tile([DKC, 1], f32, tag="bk")
        nc.sync.dma_start(bk_t[:], bk[:])
        bv_t = wpool.tile([1, DKC], f16, tag="bv")
        nc.sync.dma_start(bv_t[:], bv[:])
        bo_t = wpool.tile([1, D], f16, tag="bo")
        nc.sync.dma_start(bo_t[:], bo[:])
        ones_f = wpool.tile([1, 128], f32, tag="onesf")
        nc.gpsimd.memset(ones_f[:], 1.0)
        ones_t = wpool.tile([1, 128], f16, tag="ones")
        nc.vector.tensor_copy(ones_t[:], ones_f[:])
        onescol_f = wpool.tile([128, 1], f32, tag="onescolf")
        nc.gpsimd.memset(onescol_f[:], 1.0)
        onescol_t = wpool.tile([128, 1], f16, tag="onescol")
        nc.vector.tensor_copy(onescol_t[:], onescol_f[:])

        a2a_src = dram.tile([D, TOKC], f16, tag="a2asrc")
        a2a_dst = dram.tile([D, TOKC], f16, tag="a2adst")

        for b in range(B):
            t0 = b * S
            # ---- q/k projections -> qT_b, kT_b [128, S] (dk-major) ----
            qT_b = qkpool.tile([128, S], f16, tag=f"qT{b}")
            kT_b = qkpool.tile([128, S], f16, tag=f"kT{b}")
            for xT, w_list, bias_t, dst, nm in (
                (xqT, wq_t, bq_t, qT_b, "q"), (xkT, wk_t, bk_t, kT_b, "k"),
            ):
                xts = []
                for k in range(KT):
                    xt = xpool.tile([128, S], f16, tag="xt")
                    nc.sync.dma_start(
                        xt[:], xT[k * 128:(k + 1) * 128, t0:t0 + S])
                    xts.append(xt)
                for blk in range(SQB):
                    ps = ps_mm.tile([128, 512], f32, tag="mm")
                    for k in range(KT):
                        nc.tensor.matmul(
                            ps[:], lhsT=w_list[k][:],
                            rhs=xts[k][:, blk * 512:(blk + 1) * 512],
                            start=(k == 0), stop=(k == KT - 1),
                        )
                    nc.vector.tensor_scalar_add(
                        dst[:, blk * 512:(blk + 1) * 512], ps[:], bias_t[:, 0:1])

            # ---- v projection -> 16 tiles [128 tok, 130] ----
            v_tiles = []
            xvs = []
            for k in range(KT):
                xt = xpool.tile([128, S], f16, tag="xt")
                nc.sync.dma_start(xt[:], xvT[k * 128:(k + 1) * 128, t0:t0 + S])
                xvs.append(xt)
            for mi in range(SKT):
                ps = ps_mm.tile([128, DKC], f32, tag="mm")
                for k in range(KT):
                    nc.tensor.matmul(
                        ps[:], lhsT=xvs[k][:, mi * 128:(mi + 1) * 128],
                        rhs=wv_t[k][:], start=(k == 0), stop=False,
                    )
                nc.tensor.matmul(
                    ps[:], lhsT=ones_t[0:1, :], rhs=bv_t[:],
                    start=False, stop=True,
                )
                vt = vpool.tile([128, 130], f16, tag=f"v{b}_{mi}")
                nc.vector.tensor_copy(vt[:, 0:64], ps[:, 0:64])
                nc.vector.tensor_copy(vt[:, 65:129], ps[:, 64:128])
                nc.vector.tensor_copy(vt[:, 64:65], onescol_t[:])
                nc.vector.tensor_copy(vt[:, 129:130], onescol_t[:])
                v_tiles.append(vt)

            # ---- attention (2 heads) -> attnT_b [128, S] fp16 ----
            attnT_b = atpool.tile([128, S], f16, tag=f"attnT{b}")
            for h in range(2):
                hp = h * 64
                for sq in range(SQB):
                    qs = slice(sq * 512, (sq + 1) * 512)
                    xps = ps_acc.tile([65, 512], f32, tag="acc")
                    for sk in range(SKT):
                        sps = ps_mm.tile([128, 512], f32, tag="mm")
                        nc.tensor.matmul(
                            sps[:],
                            lhsT=kT_b[hp:hp + 64, sk * 128:(sk + 1) * 128],
                            rhs=qT_b[hp:hp + 64, qs],
                            start=True, stop=True,
                        )
                        pt = ptpool.tile([128, 512], f16, tag="pt")
                        nc.scalar.activation(pt[:], sps[:], Act.Exp, scale=0.125)
                        nc.tensor.matmul(
                            xps[:], lhsT=v_tiles[sk][:, h * 65:h * 65 + 65],
                            rhs=pt[:], start=(sk == 0), stop=(sk == SKT - 1),
                        )
                    rec = smpool.tile([1, 512], f16, tag="rec")
                    nc.vector.reciprocal(rec[:], xps[64:65, :])
                    rbp = ps_mm.tile([64, 512], f32, tag="mm")
                    nc.tensor.matmul(
                        rbp[:], lhsT=ones_t[0:1, 0:64], rhs=rec[:],
                        start=True, stop=True,
                    )
                    rb = smpool.tile([64, 512], f32, tag="rb")
                    nc.vector.tensor_copy(rb[:], rbp[:])
                    nc.vector.tensor_mul(
                        attnT_b[hp:hp + 64, qs], xps[0:64, :], rb[:],
                    )

            # ---- ship attnT blocks to a2a_src (block c = tokens of core c) --
            for blk in range(SQB):
                cg = b * SQB + blk
                nc.sync.dma_start(
                    a2a_src[cg * 128:(cg + 1) * 128, :],
                    attnT_b[:, blk * 512:(blk + 1) * 512],
                )

        # ---- AllToAll: feature slices -> token slices ----
        nc.gpsimd.collective_compute(
            "AllToAll",
            mybir.AluOpType.bypass,
            replica_groups=[list(range(NCORES))],
            ins=[a2a_src.opt()],
            outs=[a2a_dst.opt()],
        )

        # ---- output projection for my 512 tokens, full Wo ----
        recv = []
        for k in range(KT):
            t = rpool.tile([128, TOKC], f16, tag=f"recv{k}")
            nc.sync.dma_start(t[:], a2a_dst[k * 128:(k + 1) * 128, :])
            recv.append(t)
        for m in range(TOKC // 128):
            for n2 in range(2):
                ops = ps_mm.tile([128, 512], f32, tag="mm")
                for k in range(KT):
                    nc.tensor.matmul(
                        ops[:], lhsT=recv[k][:, m * 128:(m + 1) * 128],
                        rhs=wo_t[k][:, n2 * 512:(n2 + 1) * 512],
                        start=(k == 0), stop=False,
                    )
                nc.tensor.matmul(
                    ops[:], lhsT=ones_t[0:1, :],
                    rhs=bo_t[0:1, n2 * 512:(n2 + 1) * 512],
                    start=False, stop=True,
                )
                ot = opool.tile([128, 512], f32, tag="ot")
                nc.vector.tensor_copy(ot[:], ops[:])
                nc.sync.dma_start(
                    out_ext[m * 128:(m + 1) * 128,
                            n2 * 512:(n2 + 1) * 512],
                    ot[:],
                )

    nc.compile()
    return nc


def _get_nc():
    if "nc" not in _cache:
        _cache["nc"] = _build()
    return _cache["nc"]


def kernel(query, key, value, Wq, bq, Wk, bk, Wv, bv, Wo, bo, trace=False):
    from concourse.bass_utils import run_bass_kernel_spmd

    nc = _get_nc()

    q = np.ascontiguousarray(
        np.asarray(query, np.float32).reshape(TOK, D).T.astype(np.float16))
    k = np.ascontiguousarray(
        np.asarray(key, np.float32).reshape(TOK, D).T.astype(np.float16))
    v = np.ascontiguousarray(
        np.asarray(value, np.float32).reshape(TOK, D).T.astype(np.float16))
    Wq = np.asarray(Wq, np.float16)
    Wk = np.asarray(Wk, np.float16)
    Wv = np.asarray(Wv, np.float16)
    Wo = np.ascontiguousarray(np.asarray(Wo, np.float16))
    bo_h = np.ascontiguousarray(np.asarray(bo, np.float16)[None, :])

    in_maps = []
    for r in range(NCORES):
        sl = slice(r * DKC, (r + 1) * DKC)
        in_maps.append({
            "xqT": q, "xkT": k, "xvT": v,
            "wq": np.ascontiguousarray(Wq[:, sl]),
            "wk": np.ascontiguousarray(Wk[:, sl]),
            "wv": np.ascontiguousarray(Wv[:, sl]),
            "wo": Wo,
            "bq": np.ascontiguousarray(np.asarray(bq, np.float32)[sl, None]),
            "bk": np.ascontiguousarray(np.asarray(bk, np.float32)[sl, None]),
            "bv": np.ascontiguousarray(np.asarray(bv, np.float16)[None, sl]),
            "bo": bo_h,
        })

    res = run_bass_kernel_spmd(nc, in_maps, list(range(NCORES)), trace=trace)
    _cache["last_results"] = res

    out = np.concatenate([res.results[r]["out"] for r in range(NCORES)], axis=0)
    return out.reshape(B, S, D).astype(np.float32)


# revision 19
# speedup vs baseline: 2.1281x; 1.3980x over previous
"""Multi-head attention (B=2, S=2048, D=1024, H=16) on 8 NeuronCores.

Sharding: Megatron tensor parallelism. Core r owns heads 2r, 2r+1
(a 128-wide slice of D). Wq/Wk/Wv column-parallel. The output
projection is token-parallel: one AllToAll per batch exchanges
unnormalized attnT feature slices PLUS the per-head softmax sums
(130x256 fp16 blocks, ~0.5 MB) for token slices, then each core
normalizes and computes its 2x256-token output rows with the full Wo.
Host interleaves the 8 cores' token slices.

All matmul operands are fp16 (1 cyc/row on the PE, no fp32r power
throttle penalty, half the DMA); PSUM accumulation stays fp32.
Softmax normalization is deferred to the post-AllToAll phase so the
attention inner loop is a pure score->exp->PV pipeline; matmuls are
ordered so the stationary operand (kT / v tile) is reused by
consecutive instructions.

Layouts on device (per core):
  xqT/xkT/xvT : [1024, 4096]  host-transposed fp16 activations
  qT/kT       : [128, 2048]   per batch, dk-major (rows = 2 heads)
  v           : [128, 130]    16 token-tiles per batch; cols = [v_h0|1|v_h1|1]
                              (ones column makes PV emit softmax sums)
  scores^T    : psum [128 sk, 512 sq] -> exp on ACT -> pt sbuf fp16
  PV          : psum [65, 512] accumulated over 16 sk tiles; row 64 = sums
  attnT       : [128, 2048]   per batch, UNNORMALIZED fp16; sums [2, 2048]
  a2a src/dst : [1040, 256]   per batch; block c = [attnT | sums] for c's toks
  out         : [512, 1024]   fp32; rows 0:256 batch0, 256:512 batch1
"""

import sys

sys.path.insert(0, "/opt/trn_rl_repo")

import numpy as np

B, S, D, H, DK = 2, 2048, 1024, 16, 64
NCORES = 8
TOK = B * S            # 4096
DKC = D // NCORES      # 128 = 2 heads per core
TOKB = S // NCORES     # 256 tokens per core per batch
KT = D // 128          # 8 contraction tiles
SKT = S // 128         # 16 key tiles per batch
SQB = S // 512         # 4 query blocks per batch

_cache = {}


def _build():
    from contextlib import ExitStack

    from concourse import bacc
    import concourse.mybir as mybir
    import concourse.tile as tile

    f32 = mybir.dt.float32
    f16 = mybir.dt.float16
    Act = mybir.ActivationFunctionType

    nc = bacc.Bacc(
        "TRN2", target_bir_lowering=False, debug=False,
        enable_asserts=False, num_devices=NCORES,
    )

    xqT = nc.dram_tensor("xqT", [D, TOK], f16, kind="ExternalInput").ap()
    xkT = nc.dram_tensor("xkT", [D, TOK], f16, kind="ExternalInput").ap()
    xvT = nc.dram_tensor("xvT", [D, TOK], f16, kind="ExternalInput").ap()
    wq = nc.dram_tensor("wq", [D, DKC], f16, kind="ExternalInput").ap()
    wk = nc.dram_tensor("wk", [D, DKC], f16, kind="ExternalInput").ap()
    wv = nc.dram_tensor("wv", [D, DKC], f16, kind="ExternalInput").ap()
    wo = nc.dram_tensor("wo", [D, D], f16, kind="ExternalInput").ap()
    bq = nc.dram_tensor("bq", [DKC, 1], f32, kind="ExternalInput").ap()
    bk = nc.dram_tensor("bk", [DKC, 1], f32, kind="ExternalInput").ap()
    bv = nc.dram_tensor("bv", [1, DKC], f16, kind="ExternalInput").ap()
    bo = nc.dram_tensor("bo", [1, D], f16, kind="ExternalInput").ap()
    out_ext = nc.dram_tensor("out", [2 * TOKB, D], f32, kind="ExternalOutput").ap()

    with tile.TileContext(nc) as tc, ExitStack() as ctx, \
            nc.allow_low_precision("fp16 matmul operands, fp32 psum accumulate"):
        wpool = ctx.enter_context(tc.tile_pool(name="w", bufs=1))
        xpool = ctx.enter_context(tc.tile_pool(name="x", bufs=10))
        qkpool = ctx.enter_context(tc.tile_pool(name="qk", bufs=1))
        vpool = ctx.enter_context(tc.tile_pool(name="v", bufs=1))
        ptpool = ctx.enter_context(tc.tile_pool(name="pt", bufs=6))
        atpool = ctx.enter_context(tc.tile_pool(name="at", bufs=1))
        npool = ctx.enter_context(tc.tile_pool(name="norm", bufs=3))
        lnpool = ctx.enter_context(tc.tile_pool(name="lnp", bufs=2))
        opool = ctx.enter_context(tc.tile_pool(name="o", bufs=4))
        ps_mm = ctx.enter_context(tc.tile_pool(name="psmm", bufs=3, space="PSUM"))
        ps_acc = ctx.enter_context(tc.tile_pool(name="psacc", bufs=4, space="PSUM"))
        dram = ctx.enter_context(tc.tile_pool(name="dram", bufs=1, space="DRAM"))

        # ---- early weights (wo/bo deferred until after attention) ----
        wq_t, wk_t, wv_t = [], [], []
        for name, src, lst in (("wq", wq, wq_t), ("wk", wk, wk_t)):
            for k in range(KT):
                t = wpool.tile([128, DKC], f16, tag=f"{name}{k}")
                nc.sync.dma_start(t[:], src[k * 128:(k + 1) * 128, :])
                lst.append(t)
        bq_t = wpool.tile([DKC, 1], f32, tag="bq")
        nc.sync.dma_start(bq_t[:], bq[:])
        bk_t = wpool.tile([DKC, 1], f32, tag="bk")
        nc.sync.dma_start(bk_t[:], bk[:])
        for k in range(KT):
            t = wpool.tile([128, DKC], f16, tag=f"wv{k}")
            nc.sync.dma_start(t[:], wv[k * 128:(k + 1) * 128, :])
            wv_t.append(t)
        bv_t = wpool.tile([1, DKC], f16, tag="bv")
        nc.sync.dma_start(bv_t[:], bv[:])
        ones_f = wpool.tile([1, 128], f32, tag="onesf")
        nc.gpsimd.memset(ones_f[:], 1.0)
        ones_t = wpool.tile([1, 128], f16, tag="ones")
        nc.vector.tensor_copy(ones_t[:], ones_f[:])
        onescol_f = wpool.tile([128, 1], f32, tag="onescolf")
        nc.gpsimd.memset(onescol_f[:], 1.0)
        onescol_t = wpool.tile([128, 1], f16, tag="onescol")
        nc.vector.tensor_copy(onescol_t[:], onescol_f[:])

        # bv broadcast tile [128, 130] (halves at 0:64 and 65:129)
        bvb = wpool.tile([128, 130], f16, tag="bvb")
        ps_b = ps_mm.tile([128, DKC], f32, tag="mm")
        nc.tensor.matmul(ps_b[:], lhsT=ones_t[0:1, :], rhs=bv_t[:],
                         start=True, stop=True)
        nc.vector.tensor_copy(bvb[:, 0:64], ps_b[:, 0:64])
        nc.vector.tensor_copy(bvb[:, 65:129], ps_b[:, 64:128])

        a2a_src, a2a_dst = [], []
        for b in range(B):
            a2a_src_b = dram.tile([NCORES * 130, TOKB], f16, tag=f"a2asrc{b}")
            a2a_src.append(a2a_src_b)
            a2a_dst_b = dram.tile([NCORES * 130, TOKB], f16, tag=f"a2adst{b}")
            a2a_dst.append(a2a_dst_b)

        attnT = [None, None]
        sums = [None, None]

        for b in range(B):
            t0 = b * S
            # ---- q/k projections -> qT_b, kT_b [128, S] (dk-major) ----
            qT_b = qkpool.tile([128, S], f16, tag=f"qT{b}")
            kT_b = qkpool.tile([128, S], f16, tag=f"kT{b}")
            for xT, w_list, bias_t, dst in (
                (xqT, wq_t, bq_t, qT_b), (xkT, wk_t, bk_t, kT_b),
            ):
                xts = []
                for k in range(KT):
                    xt = xpool.tile([128, S], f16, tag="xt")
                    nc.sync.dma_start(
                        xt[:], xT[k * 128:(k + 1) * 128, t0:t0 + S])
                    xts.append(xt)
                pss = []
                for _blk in range(SQB):
                    ps_blk = ps_acc.tile([128, 512], f32, tag="acc")
                    pss.append(ps_blk)
                for k in range(KT):
                    for blk in range(SQB):
                        nc.tensor.matmul(
                            pss[blk][:], lhsT=w_list[k][:],
                            rhs=xts[k][:, blk * 512:(blk + 1) * 512],
                            start=(k == 0), stop=(k == KT - 1),
                        )
                for blk in range(SQB):
                    nc.vector.tensor_scalar_add(
                        dst[:, blk * 512:(blk + 1) * 512], pss[blk][:],
                        bias_t[:, 0:1])

            # ---- v projection -> 16 tiles [128 tok, 130] ----
            v_tiles = []
            xvs = []
            for k in range(KT):
                xt = xpool.tile([128, S], f16, tag="xt")
                nc.sync.dma_start(xt[:], xvT[k * 128:(k + 1) * 128, t0:t0 + S])
                xvs.append(xt)
            for mi in range(SKT):
                ps = ps_mm.tile([128, DKC], f32, tag="mm")
                for k in range(KT):
                    nc.tensor.matmul(
                        ps[:], lhsT=xvs[k][:, mi * 128:(mi + 1) * 128],
                        rhs=wv_t[k][:], start=(k == 0), stop=(k == KT - 1),
                    )
                vt = vpool.tile([128, 130], f16, tag=f"v{b}_{mi}")
                nc.vector.tensor_add(vt[:, 0:64], ps[:, 0:64], bvb[:, 0:64])
                nc.vector.tensor_add(vt[:, 65:129], ps[:, 64:128],
                                     bvb[:, 65:129])
                nc.vector.tensor_copy(vt[:, 64:65], onescol_t[:])
                nc.vector.tensor_copy(vt[:, 129:130], onescol_t[:])
                v_tiles.append(vt)

            # ---- attention (2 heads) -> unnormalized attnT_b + sums_b ----
            attnT_b = atpool.tile([128, S], f16, tag=f"attnT{b}")
            sums_b = atpool.tile([1, 2 * S], f16, tag=f"sums{b}")
            attnT[b], sums[b] = attnT_b, sums_b
            for h in range(2):
                hp = h * 64
                for sqg in range(2):      # pairs of 512-token query blocks
                    xps = []
                    for _j in range(2):
                        xp_j = ps_acc.tile([65, 512], f32, tag="acc")
                        xps.append(xp_j)
                    sqs = [slice((2 * sqg + j) * 512, (2 * sqg + j + 1) * 512)
                           for j in range(2)]
                    for sk in range(SKT):
                        sps_l = []
                        for j in range(2):
                            sps = ps_mm.tile([128, 512], f32, tag="mm")
                            nc.tensor.matmul(
                                sps[:],
                                lhsT=kT_b[hp:hp + 64, sk * 128:(sk + 1) * 128],
                                rhs=qT_b[hp:hp + 64, sqs[j]],
                                start=True, stop=True,
                            )
                            sps_l.append(sps)
                        pts = []
                        for j in range(2):
                            pt = ptpool.tile([128, 512], f16, tag="pt")
                            nc.scalar.activation(pt[:], sps_l[j][:],
                                                 Act.Exp, scale=0.125)
                            pts.append(pt)
                        for j in range(2):
                            nc.tensor.matmul(
                                xps[j][:],
                                lhsT=v_tiles[sk][:, h * 65:h * 65 + 65],
                                rhs=pts[j][:],
                                start=(sk == 0), stop=(sk == SKT - 1),
                            )
                    for j in range(2):
                        s0 = (2 * sqg + j) * 512
                        nc.vector.tensor_copy(
                            attnT_b[hp:hp + 64, sqs[j]], xps[j][0:64, :])
                        nc.vector.tensor_copy(
                            sums_b[0:1, h * S + s0:h * S + s0 + 512],
                            xps[j][64:65, :])

            # ---- ship [attnT | sums] blocks, AllToAll this batch ----
            for c in range(NCORES):
                ts = slice(c * TOKB, (c + 1) * TOKB)
                nc.sync.dma_start(
                    a2a_src[b][c * 130:c * 130 + 128, :], attnT_b[:, ts])
                for h in range(2):
                    nc.sync.dma_start(
                        a2a_src[b][c * 130 + 128 + h:c * 130 + 129 + h, :],
                        sums_b[0:1, h * S + c * TOKB:h * S + (c + 1) * TOKB])
            nc.gpsimd.collective_compute(
                "AllToAll",
                mybir.AluOpType.bypass,
                replica_groups=[list(range(NCORES))],
                ins=[a2a_src[b].opt()],
                outs=[a2a_dst[b].opt()],
            )

        # ---- late weights: full Wo + bo broadcast ----
        wo_t = []
        for k in range(KT):
            t = wpool.tile([128, D], f16, tag=f"wo{k}")
            nc.sync.dma_start(t[:], wo[k * 128:(k + 1) * 128, :])
            wo_t.append(t)
        bo_t = wpool.tile([1, D], f16, tag="bo")
        nc.sync.dma_start(bo_t[:], bo[:])
        bob = wpool.tile([128, D], f16, tag="bob")
        for n2 in range(2):
            ps = ps_mm.tile([128, 512], f32, tag="mm")
            nc.tensor.matmul(ps[:], lhsT=ones_t[0:1, :],
                             rhs=bo_t[0:1, n2 * 512:(n2 + 1) * 512],
                             start=True, stop=True)
            nc.vector.tensor_copy(bob[:, n2 * 512:(n2 + 1) * 512], ps[:])

        # ---- per batch: receive, normalize, output projection ----
        for b in range(B):
            lhsT_n = []
            for k in range(KT):
                rv = npool.tile([128, TOKB], f16, tag="rv")
                nc.sync.dma_start(rv[:], a2a_dst[b][k * 130:k * 130 + 128, :])
                rs = npool.tile([1, 2 * TOKB], f16, tag="rs")
                for h in range(2):
                    nc.sync.dma_start(
                        rs[0:1, h * TOKB:(h + 1) * TOKB],
                        a2a_dst[b][k * 130 + 128 + h:k * 130 + 129 + h, :])
                sf = npool.tile([1, 2 * TOKB], f32, tag="sf")
                nc.vector.tensor_copy(sf[:], rs[:])
                rf = npool.tile([1, 2 * TOKB], f32, tag="rf")
                nc.vector.reciprocal_approx_fast(rf[:], sf[:])
                r16 = npool.tile([1, 2 * TOKB], f16, tag="r16")
                nc.vector.tensor_copy(r16[:], rf[:])
                rb = npool.tile([128, TOKB], f16, tag="rb")
                for h in range(2):
                    rp = ps_mm.tile([64, TOKB], f32, tag="mm")
                    nc.tensor.matmul(rp[:], lhsT=ones_t[0:1, 0:64],
                                     rhs=r16[0:1, h * TOKB:(h + 1) * TOKB],
                                     start=True, stop=True)
                    nc.vector.tensor_copy(rb[h * 64:(h + 1) * 64, :], rp[:])
                ln = lnpool.tile([128, TOKB], f16, tag=f"ln{k}")
                nc.vector.tensor_mul(ln[:], rv[:], rb[:])
                lhsT_n.append(ln)
            for m2 in range(TOKB // 128):
                for n2 in range(2):
                    ops = ps_mm.tile([128, 512], f32, tag="mm")
                    for k in range(KT):
                        nc.tensor.matmul(
                            ops[:], lhsT=lhsT_n[k][:, m2 * 128:(m2 + 1) * 128],
                            rhs=wo_t[k][:, n2 * 512:(n2 + 1) * 512],
                            start=(k == 0), stop=(k == KT - 1),
                        )
                    ot = opool.tile([128, 512], f32, tag="ot")
                    nc.vector.tensor_add(
                        ot[:], ops[:], bob[:, n2 * 512:(n2 + 1) * 512])
                    nc.sync.dma_start(
                        out_ext[b * TOKB + m2 * 128:b * TOKB + (m2 + 1) * 128,
                                n2 * 512:(n2 + 1) * 512],
                        ot[:],
                    )

    nc.compile()
    return nc


def _get_nc():
    if "nc" not in _cache:
        _cache["nc"] = _build()
    return _cache["nc"]


def kernel(query, key, value, Wq, bq, Wk, bk, Wv, bv, Wo, bo, trace=False):
    from concourse.bass_utils import run_bass_kernel_spmd

    nc = _get_nc()

    q = np.ascontiguousarray(
        np.asarray(query, np.float32).reshape(TOK, D).T.astype(np.float16))
    k = np.ascontiguousarray(
        np.asarray(key, np.float32).reshape(TOK, D).T.astype(np.float16))
    v = np.ascontiguousarray(
        np.asarray(value, np.float32).reshape(TOK, D).T.astype(np.float16))
    Wq = np.asarray(Wq, np.float16)
    Wk = np.asarray(Wk, np.float16)
    Wv = np.asarray(Wv, np.float16)
    Wo = np.ascontiguousarray(np.asarray(Wo, np.float16))
    bo_h = np.ascontiguousarray(np.asarray(bo, np.float16)[None, :])

    in_maps = []
    for r in range(NCORES):
        sl = slice(r * DKC, (r + 1) * DKC)
        in_maps.append({
            "xqT": q, "xkT": k, "xvT": v,
            "wq": np.ascontiguousarray(Wq[:, sl]),
            "wk": np.ascontiguousarray(Wk[:, sl]),
            "wv": np.ascontiguousarray(Wv[:, sl]),
            "wo": Wo,
            "bq": np.ascontiguousarray(np.asarray(bq, np.float32)[sl, None]),
            "bk": np.ascontiguousarray(np.asarray(bk, np.float32)[sl, None]),
            "bv": np.ascontiguousarray(np.asarray(bv, np.float16)[None, sl]),
            "bo": bo_h,
        })

    res = run_bass_kernel_spmd(nc, in_maps, list(range(NCORES)), trace=trace)
    _cache["last_results"] = res

    out = np.empty((B, S, D), np.float32)
    for c in range(NCORES):
        o = res.results[c]["out"]
        for b in range(B):
            out[b, c * TOKB:(c + 1) * TOKB] = o[b * TOKB:(b + 1) * TOKB]
    return out
